# revision 14
# baseline (speedup 1.0000x reference)
"""Column-L2-normalization kernel for Trainium2 (8 NeuronCores, SPMD).

Computes y = x / sqrt(sum(x*x, axis=0)) for x of shape (524288, 256) fp32.

Strategy (row-sharded data parallel):
  - Each of the 8 cores gets a contiguous shard of 65536 rows (64 MB).
  - Pass 1: stream 1 MB tiles ([128 partitions x 2048 fp32], 8 rows per
    partition), square on ACT (bf16 out), fold once on DVE (fp32), reduce
    over partitions with a ones-vector matmul accumulating into PSUM.
  - Reduce the folded-row axis on DVE, AllReduce the 256-float per-column
    sums across the 8 cores, compute 1/sqrt.
  - Pass 2: re-stream the shard, multiply by the broadcast scale, write out.
    Deep load prefetch (20 tiles) hides the collective latency.
"""

import numpy as np

import concourse.bacc as bacc
import concourse.mybir as mybir
from concourse import tile
from concourse.bass_utils import run_bass_kernel_spmd

N_CORES = 8
M, C = 524288, 256
MLOC = M // N_CORES  # 65536 rows per core
P = 128  # SBUF partitions
R = 8  # rows per partition per tile
F = R * C  # free-dim elements per tile (2048)
H = F // 2  # folded free size (1024)
T = MLOC // (P * R)  # tiles per core (64)
MM = 512  # moving free dim per matmul
F32 = mybir.dt.float32
BF16 = mybir.dt.bfloat16
XBUFS = 24
NRES = 20  # tiles kept resident in SBUF between the passes


def build_nc():
    nc = bacc.Bacc("TRN2", target_bir_lowering=False, debug=False,
                   num_devices=N_CORES)
    x = nc.dram_tensor("x", [MLOC, C], F32, kind="ExternalInput")
    y = nc.dram_tensor("y", [MLOC, C], F32, kind="ExternalOutput")
    xt = x.ap().rearrange("(n p r) c -> n p (r c)", p=P, r=R)
    yt = y.ap().rearrange("(n p r) c -> n p (r c)", p=P, r=R)

    with tile.TileContext(nc) as tc:
        with (
            tc.tile_pool(name="xpool", bufs=XBUFS) as xpool,
            tc.tile_pool(name="sqpool", bufs=2) as sqpool,
            tc.tile_pool(name="small", bufs=1) as spool,
            tc.tile_pool(name="psum", bufs=1, space="PSUM") as ppool,
            tc.tile_pool(name="dram", bufs=1, space="DRAM") as dpool,
        ):
            ones = spool.tile([P, 1], BF16, tag="ones")
            nc.vector.memset(ones[:], 1.0)
            ps = ppool.tile([1, F], F32, tag="ps")
            # Warm the ACT sqrt table so the post-collective chain is short.
            warm = spool.tile([1, 4], F32, tag="warm")
            nc.vector.memset(warm[:], 1.0)
            nc.scalar.sqrt(warm[:], warm[:])

            # ---- pass 1: per-(row, column) sums of squares ----
            # The last NRES tiles stay resident in SBUF so pass 2 can skip
            # re-loading them (their squares go to a separate bf16 tile).
            resident = {}
            for i in range(T):
                xtile = xpool.tile([P, F], F32, tag="x")
                eng = nc.sync if i % 2 == 0 else nc.scalar
                eng.dma_start(xtile[:], xt[i])
                if i >= T - NRES:
                    resident[i] = xtile
                sq = sqpool.tile([P, F], BF16, tag="sq")
                nc.scalar.square(sq[:], xtile[:])
                for b in range(F // MM):
                    nc.tensor.matmul(
                        ps[:, b * MM:(b + 1) * MM],
                        ones[:],
                        sq[:, b * MM:(b + 1) * MM],
                        start=(i == 0),
                        stop=(i == T - 1),
                    )

            # ---- row-axis reduce + allreduce + rsqrt ----
            colsq = spool.tile([1, C], F32, tag="colsq")
            nc.vector.reduce_sum(
                colsq[:],
                ps[:].rearrange("p (r c) -> p c r", c=C),
                axis=mybir.AxisListType.X,
            )
            cin = dpool.tile([1, C], F32, tag="cin")
            cout = dpool.tile([1, C], F32, tag="cout")
            nc.sync.dma_start(cin[:], colsq[:])
            nc.gpsimd.collective_compute(
                "AllReduce",
                mybir.AluOpType.add,
                replica_groups=[list(range(N_CORES))],
                ins=[cin.opt()],
                outs=[cout.opt()],
            )
            gsum = spool.tile([1, C], F32, tag="gsum")
            nc.sync.dma_start(gsum[:], cout[:])
            inv = spool.tile([1, C], F32, tag="inv")
            nc.vector.reciprocal(inv[:], gsum[:])
            scl = spool.tile([1, C], F32, tag="scl")
            nc.scalar.sqrt(scl[:], inv[:])
            sclb = spool.tile([P, C], F32, tag="sclb")
            nc.gpsimd.partition_broadcast(sclb[:], scl[:])

            # ---- pass 2: scale and write out ----
            # Resident tiles first (no load needed), then re-stream the rest.
            sclb3 = sclb[:].unsqueeze(1).broadcast_to((P, R, C))
            order = list(range(T - NRES, T)) + list(range(T - NRES))
            for i in order:
                if i in resident:
                    xtile = resident[i]
                else:
                    xtile = xpool.tile([P, F], F32, tag="x")
                    nc.sync.dma_start(xtile[:], xt[i])
                v = xtile[:].rearrange("p (r c) -> p r c", c=C)
                nc.vector.tensor_mul(v, v, sclb3)
                nc.scalar.dma_start(yt[i], xtile[:])

    nc.compile()
    return nc


_NC_CACHE = None


def kernel(x) -> np.ndarray:
    global _NC_CACHE
    x = np.ascontiguousarray(np.asarray(x, dtype=np.float32))
    assert x.shape == (M, C)
    if _NC_CACHE is None:
        _NC_CACHE = build_nc()
    shards = x.reshape(N_CORES, MLOC, C)
    in_maps = [{"x": shards[i]} for i in range(N_CORES)]
    res = run_bass_kernel_spmd(_NC_CACHE, in_maps, list(range(N_CORES)))
    return np.concatenate([res.results[i]["y"] for i in range(N_CORES)], axis=0)


# revision 17
# speedup vs baseline: 1.0514x; 1.0514x over previous
"""Column-L2-normalization kernel for Trainium2 (8 NeuronCores, SPMD).

Computes y = x / sqrt(sum(x*x, axis=0)) for x of shape (524288, 256) fp32.

Strategy (row-sharded data parallel):
  - Each of the 8 cores gets a contiguous shard of 65536 rows (64 MB).
  - Pass 1: stream 1 MB tiles ([128 partitions x 2048 fp32], 8 rows per
    partition), square on ACT (bf16 out), fold once on DVE (fp32), reduce
    over partitions with a ones-vector matmul accumulating into PSUM.
  - Reduce the folded-row axis on DVE, AllReduce the 256-float per-column
    sums across the 8 cores, compute 1/sqrt.
  - Pass 2: re-stream the shard, multiply by the broadcast scale, write out.
    Deep load prefetch (20 tiles) hides the collective latency.
"""

import numpy as np

import concourse.bacc as bacc
import concourse.mybir as mybir
from concourse import tile
from concourse.bass_utils import run_bass_kernel_spmd

N_CORES = 8
M, C = 524288, 256
MLOC = M // N_CORES  # 65536 rows per core
P = 128  # SBUF partitions
R = 8  # rows per partition per tile
F = R * C  # free-dim elements per tile (2048)
H = F // 2  # folded free size (1024)
T = MLOC // (P * R)  # tiles per core (64)
MM = 512  # moving free dim per matmul
F32 = mybir.dt.float32
BF16 = mybir.dt.bfloat16
XBUFS = 24
NRES = 20  # tiles kept resident in SBUF between the passes


def build_nc():
    nc = bacc.Bacc("TRN2", target_bir_lowering=False, debug=False,
                   num_devices=N_CORES)
    x = nc.dram_tensor("x", [MLOC, C], F32, kind="ExternalInput")
    y = nc.dram_tensor("y", [MLOC, C], F32, kind="ExternalOutput")
    xt = x.ap().rearrange("(n p r) c -> n p (r c)", p=P, r=R)
    yt = y.ap().rearrange("(n p r) c -> n p (r c)", p=P, r=R)

    with tile.TileContext(nc) as tc:
        with (
            tc.tile_pool(name="xpool", bufs=XBUFS) as xpool,
            tc.tile_pool(name="sqpool", bufs=2) as sqpool,
            tc.tile_pool(name="small", bufs=1) as spool,
            tc.tile_pool(name="psum", bufs=1, space="PSUM") as ppool,
            tc.tile_pool(name="dram", bufs=1, space="DRAM") as dpool,
        ):
            ones = spool.tile([P, 1], BF16, tag="ones")
            nc.vector.memset(ones[:], 1.0)
            ps = ppool.tile([1, F], F32, tag="ps")
            # Warm the ACT sqrt table so the post-collective chain is short.
            warm = spool.tile([1, 4], F32, tag="warm")
            nc.vector.memset(warm[:], 1.0)
            nc.scalar.sqrt(warm[:], warm[:])

            # ---- pass 1: per-(row, column) sums of squares ----
            # The last NRES tiles stay resident in SBUF so pass 2 can skip
            # re-loading them (their squares go to a separate bf16 tile).
            resident = {}
            for i in range(T):
                xtile = xpool.tile([P, F], F32, tag="x")
                nc.sync.dma_start(xtile[:], xt[i])
                if i >= T - NRES:
                    resident[i] = xtile
                sq = sqpool.tile([P, F], BF16, tag="sq")
                nc.scalar.square(sq[:], xtile[:])
                for b in range(F // MM):
                    nc.tensor.matmul(
                        ps[:, b * MM:(b + 1) * MM],
                        ones[:],
                        sq[:, b * MM:(b + 1) * MM],
                        start=(i == 0),
                        stop=(i == T - 1),
                    )

            # ---- row-axis reduce + allreduce + rsqrt ----
            colsq = spool.tile([1, C], F32, tag="colsq")
            nc.vector.reduce_sum(
                colsq[:],
                ps[:].rearrange("p (r c) -> p c r", c=C),
                axis=mybir.AxisListType.X,
            )
            cin = dpool.tile([1, C], F32, tag="cin")
            cout = dpool.tile([1, C], F32, tag="cout")
            nc.gpsimd.dma_start(cin[:], colsq[:])
            nc.gpsimd.collective_compute(
                "AllReduce",
                mybir.AluOpType.add,
                replica_groups=[list(range(N_CORES))],
                ins=[cin.opt()],
                outs=[cout.opt()],
            )
            gsum = spool.tile([1, C], F32, tag="gsum")
            nc.gpsimd.dma_start(gsum[:], cout[:])
            inv = spool.tile([1, C], F32, tag="inv")
            nc.vector.reciprocal(inv[:], gsum[:])
            scl = spool.tile([1, C], F32, tag="scl")
            nc.scalar.sqrt(scl[:], inv[:])
            sclb = spool.tile([P, C], F32, tag="sclb")
            nc.gpsimd.partition_broadcast(sclb[:], scl[:])

            # ---- pass 2: scale and write out ----
            # Resident tiles first (no load needed), then re-stream the rest.
            sclb3 = sclb[:].unsqueeze(1).broadcast_to((P, R, C))
            order = list(range(T - NRES, T)) + list(range(T - NRES))
            for i in order:
                if i in resident:
                    xtile = resident[i]
                else:
                    xtile = xpool.tile([P, F], F32, tag="x")
                    nc.sync.dma_start(xtile[:], xt[i])
                v = xtile[:].rearrange("p (r c) -> p r c", c=C)
                nc.vector.tensor_mul(v, v, sclb3)
                nc.scalar.dma_start(yt[i], xtile[:])

    nc.compile()
    return nc


_NC_CACHE = None


def kernel(x) -> np.ndarray:
    global _NC_CACHE
    x = np.ascontiguousarray(np.asarray(x, dtype=np.float32))
    assert x.shape == (M, C)
    if _NC_CACHE is None:
        _NC_CACHE = build_nc()
    shards = x.reshape(N_CORES, MLOC, C)
    in_maps = [{"x": shards[i]} for i in range(N_CORES)]
    res = run_bass_kernel_spmd(_NC_CACHE, in_maps, list(range(N_CORES)))
    return np.concatenate([res.results[i]["y"] for i in range(N_CORES)], axis=0)


# revision 18
# speedup vs baseline: 1.0606x; 1.0088x over previous
"""Column-L2-normalization kernel for Trainium2 (8 NeuronCores, SPMD).

Computes y = x / sqrt(sum(x*x, axis=0)) for x of shape (524288, 256) fp32.

Strategy (row-sharded data parallel):
  - Each of the 8 cores gets a contiguous shard of 65536 rows (64 MB).
  - Pass 1: stream 1 MB tiles ([128 partitions x 2048 fp32], 8 rows per
    partition), square on ACT (bf16 out), fold once on DVE (fp32), reduce
    over partitions with a ones-vector matmul accumulating into PSUM.
  - Reduce the folded-row axis on DVE, AllReduce the 256-float per-column
    sums across the 8 cores, compute 1/sqrt.
  - Pass 2: re-stream the shard, multiply by the broadcast scale, write out.
    Deep load prefetch (20 tiles) hides the collective latency.
"""

import numpy as np

import concourse.bacc as bacc
import concourse.mybir as mybir
from concourse import tile
from concourse.bass_utils import run_bass_kernel_spmd

N_CORES = 8
M, C = 524288, 256
MLOC = M // N_CORES  # 65536 rows per core
P = 128  # SBUF partitions
R = 8  # rows per partition per tile
F = R * C  # free-dim elements per tile (2048)
H = F // 2  # folded free size (1024)
T = MLOC // (P * R)  # tiles per core (64)
MM = 512  # moving free dim per matmul
F32 = mybir.dt.float32
BF16 = mybir.dt.bfloat16
XSTREAM = 6
NRES = 18  # tiles kept resident in SBUF between the passes  # tiles kept resident in SBUF between the passes


def build_nc():
    nc = bacc.Bacc("TRN2", target_bir_lowering=False, debug=False,
                   num_devices=N_CORES)
    x = nc.dram_tensor("x", [MLOC, C], F32, kind="ExternalInput")
    y = nc.dram_tensor("y", [MLOC, C], F32, kind="ExternalOutput")
    xt = x.ap().rearrange("(n p r) c -> n p (r c)", p=P, r=R)
    yt = y.ap().rearrange("(n p r) c -> n p (r c)", p=P, r=R)

    with tile.TileContext(nc) as tc:
        with (
            tc.tile_pool(name="xs", bufs=XSTREAM) as xs_pool,
            tc.tile_pool(name="xr", bufs=NRES) as xr_pool,
            tc.tile_pool(name="sqpool", bufs=2) as sqpool,
            tc.tile_pool(name="small", bufs=1) as spool,
            tc.tile_pool(name="psum", bufs=1, space="PSUM") as ppool,
            tc.tile_pool(name="dram", bufs=1, space="DRAM") as dpool,
        ):
            ones = spool.tile([P, 1], BF16, tag="ones")
            nc.vector.memset(ones[:], 1.0)
            ps = ppool.tile([1, F], F32, tag="ps")
            # Warm the ACT sqrt table so the post-collective chain is short.
            warm = spool.tile([1, 4], F32, tag="warm")
            nc.vector.memset(warm[:], 1.0)
            nc.scalar.sqrt(warm[:], warm[:])

            # ---- pass 1: per-(row, column) sums of squares ----
            # The last NRES tiles stay resident in SBUF so pass 2 can skip
            # re-loading them (their squares go to a separate bf16 tile).
            resident = {}
            for i in range(T):
                if i >= T - NRES:
                    xtile = xr_pool.tile([P, F], F32, tag="xr")
                    resident[i] = xtile
                else:
                    xtile = xs_pool.tile([P, F], F32, tag="xs")
                nc.sync.dma_start(xtile[:], xt[i])
                sq = sqpool.tile([P, F], BF16, tag="sq")
                nc.scalar.square(sq[:], xtile[:])
                for b in range(F // MM):
                    nc.tensor.matmul(
                        ps[:, b * MM:(b + 1) * MM],
                        ones[:],
                        sq[:, b * MM:(b + 1) * MM],
                        start=(i == 0),
                        stop=(i == T - 1),
                    )

            # ---- row-axis reduce + allreduce + rsqrt ----
            colsq = spool.tile([1, C], F32, tag="colsq")
            nc.vector.reduce_sum(
                colsq[:],
                ps[:].rearrange("p (r c) -> p c r", c=C),
                axis=mybir.AxisListType.X,
            )
            cin = dpool.tile([1, C], F32, tag="cin")
            cout = dpool.tile([1, C], F32, tag="cout")
            nc.gpsimd.dma_start(cin[:], colsq[:])
            nc.gpsimd.collective_compute(
                "AllReduce",
                mybir.AluOpType.add,
                replica_groups=[list(range(N_CORES))],
                ins=[cin.opt()],
                outs=[cout.opt()],
            )
            gsum = spool.tile([1, C], F32, tag="gsum")
            nc.gpsimd.dma_start(gsum[:], cout[:])
            inv = spool.tile([1, C], F32, tag="inv")
            nc.vector.reciprocal(inv[:], gsum[:])
            scl = spool.tile([1, C], F32, tag="scl")
            nc.scalar.sqrt(scl[:], inv[:])
            sclb = spool.tile([P, C], F32, tag="sclb")
            nc.gpsimd.partition_broadcast(sclb[:], scl[:])

            # ---- pass 2: scale and write out ----
            # Resident tiles first (no load needed), then re-stream the rest.
            sclb3 = sclb[:].unsqueeze(1).broadcast_to((P, R, C))
            order = list(range(T - NRES, T)) + list(range(T - NRES))
            for i in order:
                if i in resident:
                    xtile = resident[i]
                else:
                    xtile = xs_pool.tile([P, F], F32, tag="xs")
                    nc.sync.dma_start(xtile[:], xt[i])
                v = xtile[:].rearrange("p (r c) -> p r c", c=C)
                nc.vector.tensor_mul(v, v, sclb3)
                nc.scalar.dma_start(yt[i], xtile[:])

    nc.compile()
    return nc


_NC_CACHE = None


def kernel(x) -> np.ndarray:
    global _NC_CACHE
    x = np.ascontiguousarray(np.asarray(x, dtype=np.float32))
    assert x.shape == (M, C)
    if _NC_CACHE is None:
        _NC_CACHE = build_nc()
    shards = x.reshape(N_CORES, MLOC, C)
    in_maps = [{"x": shards[i]} for i in range(N_CORES)]
    res = run_bass_kernel_spmd(_NC_CACHE, in_maps, list(range(N_CORES)))
    return np.concatenate([res.results[i]["y"] for i in range(N_CORES)], axis=0)
